# revision 4
# baseline (speedup 1.0000x reference)
"""BitLinear Trainium2 kernel v5: y = (q @ unpack2bit(W).T) * (1/s) * group_scale.

Column-parallel over 8 NeuronCores (1376 of 11008 output features each).

Design (v5 — col-tiled concurrent strips):
  1. Packed int32 weights use only their low byte — host repacks to uint8
     (1.41 MB per core HBM traffic).
  2. DVE extracts four byte-planes per dtile on u16 views:
       p0 = b & 0x0303          -> t0        (fp8 value t0 * 2^-9)
       p1 = b & 0x0C0C          -> t1 * 4
       p2 = (b >> 4) & 0x0303   -> t2
       p3 = (b >> 4) & 0x0C0C   -> t3 * 4
     Masked bytes <= 15 bitcast EXACTLY to fp8e4m3 as v * 2^-9; the 4x
     factors fold into the coefficient grids exactly (q and q/4 are both
     e4m3-exact for qh8/ql splits).
  3. TensorE runs 4 CONCURRENT column-tiled strips (tile_size 128x32,
     tile_position (0, 32p)): strip p contracts plane p against its own
     stationary coef block [128, 32] (columns = 2 activation halves x 16
     batch rows, q = qh8 + ql exact split). 4 strips stream one 512-col
     round in ~216 ns — 2x the DoubleRow rate. psum [128, chunk]:
     partition group 32p..32p+31 holds strip p's partial dot.
  4. No on-device epilogue math: psum partials are copied (f16) to SBUF by
     ACT/DVE per chunk and stored raw. Host sums the 8 partition groups
     (4 planes x 2 activation halves), applies the -S_q correction and the
     (2^9 * group_scale / s) scale in f32.
"""

import os as _os
import sys

sys.path.insert(0, "/opt/trn_rl_repo")

import numpy as np

import concourse.mybir as mybir
import concourse.tile as tile
from concourse import bacc
from concourse.bass_utils import run_bass_kernel_spmd

AluOp = mybir.AluOpType
f32 = mybir.dt.float32
f16 = mybir.dt.float16
fp8 = mybir.dt.float8e4
u8 = mybir.dt.uint8
u16 = mybir.dt.uint16
FP8NP = mybir.dt.np(fp8)

B = 16          # batch rows
K = 4096        # in_features
M = 11008       # out_features
KP = K // 4     # packed K (one byte holds 4 ternary weights)
NCORES = 8
MS = M // NCORES            # 1376 out features per core
NJT = KP // 128             # 8 j-tiles per core
NDT = NJT // 2              # 4 dtiles (2 j-tiles side by side)
W2 = 2 * MS
CHUNKS = [(0, 512), (512, 512), (1024, MS - 1024)]

N_WARM = int(_os.environ.get("WARM", "10"))
WFREE = int(_os.environ.get("WFREE", "128"))  # warm matmul free-dim width
# warm matmuls on raw (uninitialized) SBUF: their PSUM is never read
WRAW = _os.environ.get("WRAW", "1") == "1"
# store raw psum as f16 (values are ~2^-9-scaled ints, |v| << 3072; f16
# round-off is ~0.2% of out absmax vs the 2e-2 gate): halves store traffic
OUT16 = _os.environ.get("OUT16", "1") == "1"
# issue dtile 0's weight loads in the main block, hoisted before the boot
# barrier: the first data lands ~1.8 us earlier
PRE0 = _os.environ.get("PRE0", "1") == "1"
# chunk evacuation engines: chars for (c0, c1, c2); a=ACT, v=DVE
EVAC = _os.environ.get("EVAC", "ava")
# plane-op granularity for dtiles 1-3: 1 = fused both sides per op
FUSE = _os.environ.get("FUSE", "1") == "1"
# HAM-warmth bridge: filler matmuls (free dim 64) inserted after each
# dtile's rounds so PE starve gaps don't reset the 3.4us busy window
N_FILL = int(_os.environ.get("FILL", "8"))

ODT = f16 if OUT16 else f32

# plane-extraction specs: (shift, mask) per strip; values t_p * 4**(p % 2)
PLANE_OPS = [(None, 0x0303), (None, 0x0C0C), (4, 0x0303), (4, 0x0C0C)]


def _plane_op(nc, dst16, src16, p):
    sh, mask = PLANE_OPS[p]
    if sh is None:
        return nc.vector.tensor_scalar(dst16, src16, mask, None, AluOp.bitwise_and)
    return nc.vector.tensor_scalar(
        dst16, src16, sh, mask, AluOp.logical_shift_right, AluOp.bitwise_and
    )


def build_kernel_body(tc, pT_d, coef_d, out_d, pre0=None, warm_raw=None):
    nc = tc.nc
    with (
        tc.tile_pool(name="sbuf", bufs=1) as pool,
        tc.tile_pool(name="const", bufs=1) as cpool,
        tc.tile_pool(name="psum", bufs=1, space="PSUM") as psum_pool,
    ):
        psums = [
            psum_pool.tile([128, ln], f32, tag=f"psum{ci}", name=f"psum{ci}")
            for ci, (_, ln) in enumerate(CHUNKS)
        ]

        # weight loads: side-0 halves on sync HWDGE, side-1 on scalar HWDGE
        coef_sb = cpool.tile([128, NJT, 4, 32], fp8, tag="coef")
        p8s = []
        for dt in range(NDT):
            rows = slice(dt * 128, (dt + 1) * 128)
            if dt == 0 and pre0 is not None:
                p8s.append(pre0[0])
                nc.sync.dma_start(coef_sb[:, 0:4], coef_d[:, 0:4])
                nc.scalar.dma_start(coef_sb[:, 4:8], coef_d[:, 4:8])
                continue
            p8 = pool.tile([128, W2], u8, tag=f"p8_{dt}", name=f"p8_{dt}")
            nc.sync.dma_start(p8[:, :MS], pT_d[rows, :MS])
            nc.scalar.dma_start(p8[:, MS:], pT_d[rows, MS:])
            p8s.append(p8)
            if dt == 0:
                nc.sync.dma_start(coef_sb[:, 0:4], coef_d[:, 0:4])
                nc.scalar.dma_start(coef_sb[:, 4:8], coef_d[:, 4:8])

        # PE clock warmup (col-tiled mode, same tile_size as the real mms)
        if warm_raw is not None:
            wl, wr = warm_raw
        else:
            wl = cpool.tile([128, 32], fp8, tag="wl")
            wr = cpool.tile([128, WFREE], fp8, tag="wr")
            nc.vector.memset(wl[:], 1.0)
            nc.vector.memset(wr[:], 1.0)
        warm = psum_pool.tile([32, WFREE], f32, tag="warm")
        for _ in range(N_WARM):
            nc.tensor.matmul(
                warm[:], wl[:], wr[:], start=True, stop=True,
                tile_position=(0, 0),
            )

        planes = [
            pool.tile([128, 4, 2, MS], u8, tag=f"pl_{dt}", name=f"pl_{dt}")
            for dt in range(NDT)
        ]

        # plane extraction + matmul rounds, dtile-major
        for dt in range(NDT):
            p8 = p8s[dt]
            pl = planes[dt]
            if dt == 0 and pre0 is not None:
                # per-side ops gated on the pre-context DMA semaphores
                for side in range(2):
                    gate = pre0[1][side]
                    for p in range(4):
                        op = _plane_op(
                            nc,
                            pl[:, p, side, :].bitcast(u16),
                            p8[:, side * MS : (side + 1) * MS].bitcast(u16),
                            p,
                        )
                        pre0[2].append((op, gate[0], gate[1]))
            elif FUSE:
                for p in range(4):
                    _plane_op(
                        nc, pl[:, p, :, :].bitcast(u16), p8[:].bitcast(u16), p
                    )
            else:
                for side in range(2):
                    for p in range(4):
                        _plane_op(
                            nc,
                            pl[:, p, side, :].bitcast(u16),
                            p8[:, side * MS : (side + 1) * MS].bitcast(u16),
                            p,
                        )

            for side in range(2):
                jt = 2 * dt + side
                final = jt == NJT - 1
                for ci, (off, ln) in enumerate(CHUNKS):
                    for p in range(4):
                        nc.tensor.matmul(
                            psums[ci][32 * p : 32 * (p + 1), :],
                            coef_sb[:, jt, p, :],
                            pl[:, p, side, off : off + ln].bitcast(fp8),
                            start=(jt == 0),
                            stop=final,
                            tile_position=(0, 32 * p),
                        )
            if dt < NDT - 1:
                for _ in range(N_FILL):
                    nc.tensor.matmul(
                        warm[:, :64], wl[:], wr[:, :64],
                        start=True, stop=True, tile_position=(0, 0),
                    )

        # evacuation: raw psum -> sbuf (dtype-converted), then store.
        for ci, (off, ln) in enumerate(CHUNKS):
            osb = pool.tile([128, ln], ODT, tag=f"osb{ci}", name=f"osb{ci}")
            if EVAC[ci] == "a":
                nc.scalar.copy(osb[:], psums[ci][:])
            else:
                nc.vector.tensor_copy(osb[:], psums[ci][:])
            nc.sync.dma_start(out_d[:64, off : off + ln], osb[:64])
            nc.scalar.dma_start(out_d[64:, off : off + ln], osb[64:])


def build_nc():
    nc = bacc.Bacc("TRN2", target_bir_lowering=False)
    pT_d = nc.dram_tensor("pT", [KP // 2, W2], u8, kind="ExternalInput")
    coef_d = nc.dram_tensor("coef", [128, NJT, 4, 32], fp8, kind="ExternalInput")
    out_d = nc.dram_tensor("out", [128, MS], ODT, kind="ExternalOutput")
    pre0 = None
    if PRE0:
        # dtile 0 loads issued in the main block, ahead of the tile-entry
        # drain; consumers in the body wait on the completion semaphores
        p8_0 = nc.alloc_sbuf_tensor("p8_0raw", [128, W2], u8)
        sem_h0 = nc.alloc_semaphore("w0h0")
        sem_h1 = nc.alloc_semaphore("w0h1")
        d0 = nc.sync.dma_start(p8_0.ap()[:, :MS], pT_d[0:128, :MS]).then_inc(sem_h0, 16)
        d1 = nc.scalar.dma_start(p8_0.ap()[:, MS:], pT_d[0:128, MS:]).then_inc(sem_h1, 16)
        # hoist the two descriptors to the very front of the main block
        mainblk = nc.m.functions[0].blocks[0]
        hoisted = [d0.ins, d1.ins]
        ids = {id(i) for i in hoisted}
        rest = [i for i in mainblk.instructions if id(i) not in ids]
        for i in reversed(hoisted):
            rest.insert(0, i)
        try:
            mainblk.set_instructions(rest)
        except AttributeError:
            while len(mainblk.instructions):
                del mainblk.instructions[0]
            for i in rest:
                mainblk.add_instruction(i)
        pre0 = (p8_0.ap(), [(sem_h0, 16), (sem_h1, 16)], [])
    warm_raw = None
    if WRAW:
        wl_t = nc.alloc_sbuf_tensor("wlraw", [128, 32], fp8)
        wr_t = nc.alloc_sbuf_tensor("wrraw", [128, WFREE], fp8)
        warm_raw = (wl_t.ap(), wr_t.ap())
    with tile.TileContext(nc) as tc:
        build_kernel_body(tc, pT_d, coef_d, out_d, pre0=pre0, warm_raw=warm_raw)
    if pre0 is not None:
        # gate each side's plane ops on the pre-context DMA completion:
        # splice a standalone wait instruction into the scheduled block
        # just before the first gated op of each side
        import concourse.bass as _bass

        by_gate = {}
        for op, sem, val in pre0[2]:
            by_gate.setdefault((sem, val), []).append(op.ins)
        for (sem, val), insts in by_gate.items():
            targets = set(id(i) for i in insts)
            for f in nc.m.functions:
                for blk in f.blocks:
                    idxs = [
                        i for i, inst in enumerate(blk.instructions)
                        if id(inst) in targets
                    ]
                    if not idxs:
                        continue
                    ev = mybir.InstEventSemaphore(
                        name=nc.get_next_instruction_name(), ins=[], outs=[]
                    )
                    ev.engine = mybir.EngineType.DVE
                    _bass.BassInstruction(ev).wait_op(sem, val, "sem-ge")
                    nc.register_instruction(ev)
                    blk.instructions.insert(min(idxs), ev)
    nc.compile()
    return nc


def prepare_inputs(input, weight_packed, weight_scale):
    """Host-side shard/layout prep. Returns per-core input maps."""
    inp = np.asarray(input, dtype=np.float32)
    wp = np.asarray(weight_packed, dtype=np.int32)
    ws = np.asarray(weight_scale, dtype=np.float32)

    # activation quantization (matches reference: f32, round-half-even)
    amax = np.maximum(np.max(np.abs(inp), axis=-1, keepdims=True), np.float32(1e-5))
    s = np.float32(127.0) / amax                          # [B,1] f32
    q = np.clip(np.round(inp * s), -128.0, 127.0).astype(np.float32)  # [B,K]

    # split q = qh8 + ql, both parts exactly representable in e4m3:
    # qh8 on the step-8 grid (|qh8| <= 128), ql in [-4, 4]
    qh8 = 8.0 * np.round(q * 0.125)
    ql = q - qh8
    assert np.abs(qh8).max() <= 128 and np.abs(ql).max() <= 4

    # coef layout [k=128, jt, p, col] with col = half*16 + b:
    #   value = qX_b[4*(jt*128 + k) + p] * (1 if p in {0,2} else 1/4)
    qs = np.stack([qh8, ql], axis=0)                  # [half, B, K]
    qsv = qs.reshape(2, B, NJT, 128, 4)               # [half, b, jt, k, p]
    coef = np.ascontiguousarray(
        qsv.transpose(3, 2, 4, 0, 1)                  # [k, jt, p, half, b]
    ).reshape(128, NJT, 4, 2 * B)
    pscale = np.array([1.0, 0.25, 1.0, 0.25], np.float32)[None, None, :, None]
    coef = coef * pscale
    coef_sb = coef.astype(FP8NP)
    assert np.array_equal(coef_sb.astype(np.float32), coef)

    sq = q.sum(axis=-1)                               # [B]
    srecip = (np.float32(2.0**9) / s[:, 0]).astype(np.float32)  # [B]

    wp_u8 = wp.astype(np.uint8)
    in_maps = []
    post = []
    for core in range(NCORES):
        m0 = core * MS
        # [KP, MS] -> double-width [KP/2, 2*MS]: row p of block dt holds
        # j = dt*256+p (cols 0:MS) and j = dt*256+128+p (cols MS:2*MS)
        pT_core = np.ascontiguousarray(
            wp_u8[m0 : m0 + MS]
            .T.reshape(NDT, 2, 128, MS)
            .transpose(0, 2, 1, 3)
            .reshape(KP // 2, W2)
        )
        gs = ws[(m0 // (M // ws.shape[0]))]
        in_maps.append({"pT": pT_core, "coef": coef_sb})
        post.append(gs)
    return in_maps, sq, srecip, post


_NC_CACHE = {}


def run(input, weight_packed, weight_scale, trace=False):
    if "nc" not in _NC_CACHE:
        _NC_CACHE["nc"] = build_nc()
    nc = _NC_CACHE["nc"]
    in_maps, sq, srecip, gss = prepare_inputs(input, weight_packed, weight_scale)
    res = run_bass_kernel_spmd(nc, in_maps, core_ids=list(range(NCORES)), trace=trace)
    outs = []
    for core, r in enumerate(res.results):
        H = r["out"].astype(np.float32)              # [128, MS]
        # partition layout: 32p + half*16 + b
        total = H.reshape(4, 2, B, MS).sum(axis=(0, 1))   # [B, MS]
        out_core = (total * np.float32(2.0**9) - sq[:, None]) * (
            srecip[:, None] / np.float32(2.0**9) * gss[core]
        )
        outs.append(out_core.astype(np.float32))
    return np.concatenate(outs, axis=1), res


def kernel(**inputs):
    out, _ = run(
        inputs["input"], inputs["weight_packed"], inputs["weight_scale"], trace=False
    )
    return out
